# revision 2
# baseline (speedup 1.0000x reference)
"""Trainium2 Bass kernel for a binarized 4-layer MLP (eval mode).

Reference computation (per row of x [B=16384, 784]):
  h1 = x @ sign(w1).T + b1;  s1 = sign(bn1(h1))        (clip doesn't change sign)
  h2 = s1 @ sign(w2).T + b2; s2 = sign(bn2(h2))
  h3 = s2 @ sign(w3).T + b3; y3 = clip(bn3(h3), -1, 1)
  z  = y3 @ w4.T + b4;       out = log_softmax(z)

Sharding: pure data-parallel over the batch across 8 NeuronCores
(weights replicated, no collectives).

Numerics:
  - L1: x is split ON HOST into xa = fp16(x) plus an fp8e4m3 residual
    stream xr = e4m3((x - xa) * 2^9), contracted against +-2^-9 weights
    (exactly representable e4m3 subnormals) with fp8 DoubleRow matmuls.
    7 fp16 matmuls + 4 DR matmuls per m-tile replace the 14 fp16
    matmuls of a two-stream fp16 split: ~2^-16.7 effective x precision,
    final rel err ~7.6e-3 (measured in f64 simulation) vs 2e-2 budget.
  - L2/L3: both operands are exactly +-1/0 in fp8e4 -> DoubleRow fp8
    matmuls produce bit-exact integer sums in fp32 PSUM.
  - L4 runs transposed (w4 [128,10] stationary, h3 moving) so the PE
    streams 256-col moving operands instead of 10-col ones; [10, b]
    logit tiles are PE-transposed back for the row-major epilogue.
  - BN + bias folding: bn(h + b) = A*h + C with A = g*rsqrt(v+eps),
    C = A*(b - m) + beta, applied per-partition by the Sign/Identity
    activations (fp32 internally).
"""

import sys

if "/opt/trn_rl_repo" not in sys.path:
    sys.path.insert(0, "/opt/trn_rl_repo")

import numpy as np

D_IN, H1, H2, H3, NCLS = 784, 3072, 1536, 768, 10
B, NCORES = 16384, 8
BC = B // NCORES          # batch rows per core
NB = 256                  # batch columns processed per chunk (L2-L4)
KP = 112                  # L1 fp16 k-tile partition size (784 = 7 * 112)
K1T = D_IN // KP          # 7
KR = 1024                 # L1 residual k rows (784 padded to 8 * 128)
KRT = KR // 128           # 8
KRP = KRT // 2            # 4 DoubleRow pair iterations
M1, M2, M3 = H1 // 128, H2 // 128, H3 // 128   # 24, 12, 6
K2P, K3P = H1 // 256, H2 // 256                # DoubleRow k-pair iters: 12, 6
K4T = H3 // 128                                # 6
BN_EPS = 1e-5
RSC = 512.0               # residual scale 2^9; weights are +-2^-9

_cached = {}


def _build(bc):
    import concourse.bacc as bacc
    import concourse.mybir as mybir
    import concourse.tile as tile

    dt = mybir.dt
    AF = mybir.ActivationFunctionType
    PM = mybir.MatmulPerfMode
    ALU = mybir.AluOpType

    assert bc % NB == 0 and NB % 128 == 0
    gbts = bc // 128  # output row-tiles per core

    nc = bacc.Bacc("TRN2", target_bir_lowering=False, debug=False,
                   num_devices=NCORES)

    xa16 = nc.declare_dram_parameter("xa16", [D_IN, bc], dt.float16, isOutput=False)
    xr8 = nc.declare_dram_parameter("xr8", [KR, bc], dt.float8e4, isOutput=False)
    w1t = nc.declare_dram_parameter("w1t", [D_IN, H1], dt.float16, isOutput=False)
    w1r = nc.declare_dram_parameter("w1r", [KR, H1], dt.float8e4, isOutput=False)
    w2t = nc.declare_dram_parameter("w2t", [H1, H2], dt.float8e4, isOutput=False)
    w3t = nc.declare_dram_parameter("w3t", [H2, H3], dt.float8e4, isOutput=False)
    w4t = nc.declare_dram_parameter("w4t", [H3, NCLS], dt.bfloat16, isOutput=False)
    a1s = nc.declare_dram_parameter("a1s", [128, M1], dt.float32, isOutput=False)
    c1s = nc.declare_dram_parameter("c1s", [128, M1], dt.float32, isOutput=False)
    a2s = nc.declare_dram_parameter("a2s", [128, M2], dt.float32, isOutput=False)
    c2s = nc.declare_dram_parameter("c2s", [128, M2], dt.float32, isOutput=False)
    a3s = nc.declare_dram_parameter("a3s", [128, M3], dt.float32, isOutput=False)
    c3s = nc.declare_dram_parameter("c3s", [128, M3], dt.float32, isOutput=False)
    b4s = nc.declare_dram_parameter("b4s", [128, NCLS], dt.float32, isOutput=False)
    id10 = nc.declare_dram_parameter("id10", [NCLS, NCLS], dt.float32,
                                     isOutput=False)
    out = nc.declare_dram_parameter("out", [bc, NCLS], dt.float32, isOutput=True)

    with tile.TileContext(nc) as tc, \
            tc.tile_pool(name="wts", bufs=1) as wp, \
            tc.tile_pool(name="xin", bufs=2) as xp, \
            tc.tile_pool(name="act", bufs=2) as ap_, \
            tc.tile_pool(name="eps", bufs=2) as ep, \
            tc.tile_pool(name="ps", bufs=4, space="PSUM") as ps, \
            tc.tile_pool(name="ps4", bufs=2, space="PSUM") as ps4:

        # ---- startup-critical transfers first: consts, chunk-0 x streams,
        # first residual-weight block, then w1.  w2/w3 streams are
        # dependency-chained onto chunk-0 compute milestones below so they
        # don't steal HBM bandwidth at startup.
        a1sb = wp.tile([128, M1], dt.float32, tag="a1")
        c1sb = wp.tile([128, M1], dt.float32, tag="c1")
        a2sb = wp.tile([128, M2], dt.float32, tag="a2")
        c2sb = wp.tile([128, M2], dt.float32, tag="c2")
        a3sb = wp.tile([128, M3], dt.float32, tag="a3")
        c3sb = wp.tile([128, M3], dt.float32, tag="c3")
        b4sb = wp.tile([128, NCLS], dt.float32, tag="b4")
        id10sb = wp.tile([NCLS, NCLS], dt.float32, tag="id10")
        for sb, drh in ((a1sb, a1s), (c1sb, c1s), (a2sb, a2s), (c2sb, c2s),
                        (a3sb, a3s), (c3sb, c3s), (b4sb, b4s), (id10sb, id10)):
            nc.sync.dma_start(sb[:], drh[:])

        # L1 runs on wide batch groups (W columns); L2-L4 iterate over
        # NB-column halves of each group.
        W = 2 * NB if bc % (2 * NB) == 0 else NB
        ngroups = bc // W
        halves = W // NB
        NBLK = M1 // 4            # m-tile blocks of 4 (PSUM rotation depth)

        def load_x(g):
            # returns (list of per-k-tile fp16 APs, residual [128,KRT,W] AP,
            #          first dma)
            cs = slice(g * W, (g + 1) * W)
            if g == 0:
                # group 0 is startup-latency critical: separate tiles per
                # k-tile so each matmul depends only on its own k-tile's DMA
                xas = []
                xdma = None
                for k in range(K1T):
                    xak = xp.tile([KP, W], dt.float16, tag=f"xa{k}", bufs=1,
                                  name=f"xa{k}")
                    d = nc.sync.dma_start(xak[:],
                                          xa16[k * KP:(k + 1) * KP, cs])
                    xdma = xdma or d
                    xas.append(xak)
                xrg = xp.tile([128, KRT, W], dt.float8e4, tag="xr0", bufs=1,
                              name="xr0")
                nc.sync.dma_start(
                    xrg[:], xr8.ap()[:, cs].rearrange("(kt p) b -> p kt b",
                                                      p=128))
                return xas, xrg, xdma
            xag = xp.tile([KP, K1T, W], dt.float16, tag="xa")
            xdma = nc.sync.dma_start(
                xag[:], xa16.ap()[:, cs].rearrange("(kt p) b -> p kt b", p=KP))
            xrg = xp.tile([128, KRT, W], dt.float8e4, tag="xr")
            xrdma = nc.sync.dma_start(
                xrg[:], xr8.ap()[:, cs].rearrange("(kt p) b -> p kt b", p=128))
            return ([xag[:, k, :] for k in range(K1T)], xrg, xdma)

        x0 = load_x(0)

        # first residual-weight block before the w1 stream: block 0's DR
        # matmuls follow right behind its fp16 matmuls
        w1rb = [None] * NBLK
        w1rb[0] = wp.tile([128, KRT, 512], dt.float8e4, tag="w1r0",
                          name="w1r0")
        nc.sync.dma_start(
            w1rb[0][:], w1r.ap()[:, 0:512].rearrange("(kt p) h -> p kt h",
                                                     p=128))

        # Per-k-tile w1 tiles so matmuls depend only on their own transfer.
        w1k = []
        for kt in range(K1T):
            wk = wp.tile([KP, H1], dt.float16, tag=f"w1_{kt}", name=f"w1_{kt}")
            nc.sync.dma_start(wk[:], w1t[kt * KP:(kt + 1) * KP, :])
            w1k.append(wk)

        for mg in range(1, NBLK):
            w1rb[mg] = wp.tile([128, KRT, 512], dt.float8e4, tag=f"w1r{mg}",
                               name=f"w1r{mg}")
            nc.sync.dma_start(
                w1rb[mg][:],
                w1r.ap()[:, mg * 512:(mg + 1) * 512].rearrange(
                    "(kt p) h -> p kt h", p=128))

        w4sb = wp.tile([128, K4T, NCLS], dt.bfloat16, tag="w4")
        nc.sync.dma_start(w4sb[:], w4t.ap().rearrange("(kt p) n -> p kt n", p=128))

        w2sb = wp.tile([128, 2 * K2P, H2], dt.float8e4, tag="w2")
        w2_dmas = [
            nc.sync.dma_start(w2sb[:, kt, :], w2t[kt * 128:(kt + 1) * 128, :])
            for kt in range(2 * K2P)
        ]
        w3sb = wp.tile([128, 2 * K3P, H3], dt.float8e4, tag="w3")
        w3_dmas = [
            nc.sync.dma_start(w3sb[:, kt, :], w3t[kt * 128:(kt + 1) * 128, :])
            for kt in range(2 * K3P)
        ]

        zout = wp.tile([128, gbts, NCLS], dt.float32, tag="zout")
        ssum = wp.tile([128, gbts], dt.float32, tag="ssum")
        lsum = wp.tile([128, gbts], dt.float32, tag="lsum")

        def emit_epilogue(lo, hi):
            # log_softmax over the free dim; |z| is small so no max-shift
            for g in range(lo, hi):
                e = ep.tile([128, NCLS], dt.float32, tag="e")
                nc.scalar.activation(e[:], zout[:, g, :], AF.Exp,
                                     accum_out=ssum[:, g:g + 1])
            nc.scalar.activation(lsum[:, lo:hi], ssum[:, lo:hi], AF.Ln)
            for g in range(lo, hi):
                nc.vector.tensor_scalar(zout[:, g, :], zout[:, g, :],
                                        lsum[:, g:g + 1], None,
                                        op0=ALU.subtract)
            nc.sync.dma_start(
                out.ap()[lo * 128:hi * 128, :].rearrange("(g p) n -> p g n",
                                                         p=128),
                zout[:, lo:hi, :])

        prev_act0 = None
        for g in range(ngroups):
            xa, xrg, _ = x0 if g == 0 else (None, None, None)
            if g != 0:
                xa, xrg, xdma = load_x(g)
                if prev_act0 is not None:
                    # keep ~one group of x lookahead; don't fight the
                    # startup transfers
                    tile.add_dep_helper(xdma.ins, prev_act0.ins, sync=True,
                                        reason="x prefetch staging")

            # ---- L1: [784 -> 3072] = 7 fp16 matmuls + 4 fp8 DoubleRow
            # residual matmuls per m-tile, blocks of 4 m-tiles over 4 PSUM
            # banks, kt-outer so the PE consumes each w1/x k-tile as its
            # DMA lands at startup.
            h1sb = ap_.tile([128, 2 * K2P, W], dt.float8e4, tag="h1")

            def l1_sign(mt, pt):
                act = nc.scalar.activation(h1sb[:, mt, :], pt[:], AF.Sign,
                                           bias=c1sb[:, mt:mt + 1],
                                           scale=a1sb[:, mt:mt + 1])
                if g == 0:
                    # stage w2/w3 weight streams behind group-0 L1 progress
                    # so they don't starve the startup transfers
                    for wd_list, base in ((w2_dmas, 0), (w3_dmas, M1 // 2)):
                        for kt2, wd in enumerate(wd_list):
                            if base + kt2 // 2 == mt:
                                tile.add_dep_helper(
                                    wd.ins, act.ins, sync=True,
                                    reason="weight stream staging")
                if mt == 0:
                    return act
                return None

            for mg in range(NBLK):
                pts = [ps.tile([128, W], dt.float32, tag="ps",
                               name=f"pt{i}") for i in range(4)]
                for kt in range(K1T):
                    for i in range(4):
                        mt = mg * 4 + i
                        lhs = w1k[kt][:, mt * 128:(mt + 1) * 128]
                        nc.tensor.matmul(pts[i][:], lhs, xa[kt][:],
                                         start=(kt == 0), stop=False)
                for i in range(4):
                    mt = mg * 4 + i
                    for rp in range(KRP):
                        nc.tensor.matmul(
                            pts[i][:],
                            w1rb[mg][:, 2 * rp:2 * rp + 2,
                                     i * 128:(i + 1) * 128],
                            xrg[:, 2 * rp:2 * rp + 2, :],
                            start=False, stop=(rp == KRP - 1),
                            perf_mode=PM.DoubleRow)
                    a = l1_sign(mt, pts[i])
                    prev_act0 = a or prev_act0

            for h in range(halves):
                hs = slice(h * NB, (h + 1) * NB)
                # ---- L2: [3072 -> 1536], fp8 DoubleRow
                h2sb = ap_.tile([128, 2 * K3P, NB], dt.float8e4, tag="h2")
                for mt in range(M2):
                    pt = ps.tile([128, NB], dt.float32, tag="ps")
                    for kp in range(K2P):
                        nc.tensor.matmul(
                            pt[:],
                            w2sb[:, 2 * kp:2 * kp + 2, mt * 128:(mt + 1) * 128],
                            h1sb[:, 2 * kp:2 * kp + 2, hs],
                            start=(kp == 0), stop=(kp == K2P - 1),
                            perf_mode=PM.DoubleRow)
                    nc.scalar.activation(h2sb[:, mt, :], pt[:], AF.Sign,
                                         bias=c2sb[:, mt:mt + 1],
                                         scale=a2sb[:, mt:mt + 1])

                # ---- L3: [1536 -> 768], fp8 DoubleRow; output clipped bf16
                h3c = ap_.tile([128, K4T, NB], dt.bfloat16, tag="h3")
                for mt in range(M3):
                    pt = ps.tile([128, NB], dt.float32, tag="ps")
                    for kp in range(K3P):
                        nc.tensor.matmul(
                            pt[:],
                            w3sb[:, 2 * kp:2 * kp + 2, mt * 128:(mt + 1) * 128],
                            h2sb[:, 2 * kp:2 * kp + 2, :],
                            start=(kp == 0), stop=(kp == K3P - 1),
                            perf_mode=PM.DoubleRow)
                    nc.vector.tensor_scalar(h3c[:, mt, :], pt[:],
                                            a3sb[:, mt:mt + 1],
                                            c3sb[:, mt:mt + 1],
                                            op0=ALU.mult, op1=ALU.add)
                    nc.vector.tensor_scalar(h3c[:, mt, :], h3c[:, mt, :],
                                            1.0, -1.0, op0=ALU.min,
                                            op1=ALU.max)

                # ---- L4: z.T = w4 @ y3.T, w4 stationary [128,10], h3
                # moving [128,NB]; then PE-transpose [10,128] chunks back
                # to row-major and add b4.
                zp = ps4.tile([NCLS, NB], dt.float32, tag="zp")
                for kt in range(K4T):
                    nc.tensor.matmul(zp[:], w4sb[:, kt, :], h3c[:, kt, :],
                                     start=(kt == 0), stop=(kt == K4T - 1))
                zs = ep.tile([NCLS, NB], dt.float32, tag="zs")
                nc.vector.tensor_copy(zs[:], zp[:])
                for bt in range(NB // 128):
                    gbt = (g * halves + h) * (NB // 128) + bt
                    ztp = ps4.tile([128, NCLS], dt.float32, tag="zt")
                    nc.tensor.transpose(ztp[:],
                                        zs[:, bt * 128:(bt + 1) * 128],
                                        id10sb[:])
                    nc.vector.tensor_add(zout[:, gbt, :], ztp[:], b4sb[:])

                if (g == ngroups - 1 and ngroups >= 2 and halves == 2
                        and h == 0):
                    # first half of the last group: epilogue overlaps the
                    # second half's matmuls
                    emit_epilogue(gbts - 4, gbts - 2)

            if g == ngroups - 2:
                # bulk of the log-softmax epilogue hides under the last
                # group's matmuls; only the final row-tiles run in the tail
                emit_epilogue(0, (g + 1) * W // 128)

        if ngroups >= 2 and halves == 2:
            emit_epilogue(gbts - 2, gbts)
        elif ngroups >= 2:
            emit_epilogue((ngroups - 1) * W // 128, gbts)
        else:
            emit_epilogue(0, gbts)

    nc.finalize()
    return nc


def _prep(x, w1, b1, w2, b2, w3, b3, w4, b4,
          g1, be1, m1, v1, g2, be2, m2, v2, g3, be3, m3, v3):
    """Host-side layout prep: transposes, binarized weight casts, BN folds,
    and the fp16 + scaled-e4m3-residual split of x."""
    import concourse.mybir as mybir
    f8 = mybir.dt.np(mybir.dt.float8e4)

    def fold(g, be, m, v, b):
        a = (g / np.sqrt(v + np.float32(BN_EPS))).astype(np.float32)
        c = (a * (b - m) + be).astype(np.float32)
        return a, c

    a1, c1 = fold(g1, be1, m1, v1, b1)
    a2, c2 = fold(g2, be2, m2, v2, b2)
    a3, c3 = fold(g3, be3, m3, v3, b3)

    def cols(v, mtiles):
        return np.ascontiguousarray(v.reshape(mtiles, 128).T)

    s1 = np.sign(w1).T  # [784, 3072]
    w1rp = np.zeros((KR, H1), dtype=f8)
    w1rp[:D_IN] = (s1 / RSC).astype(f8)

    pre = dict(
        w1t=np.ascontiguousarray(s1).astype(np.float16),
        w1r=np.ascontiguousarray(w1rp),
        w2t=np.ascontiguousarray(np.sign(w2).T).astype(f8),
        w3t=np.ascontiguousarray(np.sign(w3).T).astype(f8),
        w4t=np.ascontiguousarray(w4.T).astype(mybir.dt.np(mybir.dt.bfloat16)),
        a1s=cols(a1, M1), c1s=cols(c1, M1),
        a2s=cols(a2, M2), c2s=cols(c2, M2),
        a3s=cols(a3, M3), c3s=cols(c3, M3),
        b4s=np.ascontiguousarray(np.tile(b4.astype(np.float32), (128, 1))),
        id10=np.eye(NCLS, dtype=np.float32),
    )
    xt = np.ascontiguousarray(x.T.astype(np.float32))  # [784, B]
    xa = xt.astype(np.float16)
    xr = np.zeros((KR, x.shape[0]), dtype=f8)
    xr[:D_IN] = ((xt - xa.astype(np.float32)) * np.float32(RSC)).astype(f8)
    return pre, xa, xr


def run(inputs, **spmd_kwargs):
    from concourse.bass_utils import run_bass_kernel_spmd

    if "nc" not in _cached:
        _cached["nc"] = _build(BC)
    nc = _cached["nc"]

    inputs = {k: np.asarray(v) for k, v in inputs.items()}
    pre, xa, xr = _prep(**inputs)

    in_maps = []
    for core in range(NCORES):
        m = dict(pre)
        m["xa16"] = np.ascontiguousarray(xa[:, core * BC:(core + 1) * BC])
        m["xr8"] = np.ascontiguousarray(xr[:, core * BC:(core + 1) * BC])
        in_maps.append(m)

    res = run_bass_kernel_spmd(nc, in_maps, list(range(NCORES)), **spmd_kwargs)
    outs = [res.results[i]["out"] for i in range(NCORES)]
    return res, np.concatenate(outs, axis=0).astype(np.float32)


def kernel(**inputs):
    return run(inputs)[1]


# revision 10
# speedup vs baseline: 1.2261x; 1.2261x over previous
"""Trainium2 Bass kernel for a binarized 4-layer MLP (eval mode).

Reference computation (per row of x [B=16384, 784]):
  h1 = x @ sign(w1).T + b1;  s1 = sign(bn1(h1))        (clip doesn't change sign)
  h2 = s1 @ sign(w2).T + b2; s2 = sign(bn2(h2))
  h3 = s2 @ sign(w3).T + b3; y3 = clip(bn3(h3), -1, 1)
  z  = y3 @ w4.T + b4;       out = log_softmax(z)

Sharding: pure data-parallel over the batch across 8 NeuronCores
(weights replicated, no collectives).

Numerics:
  - L1: x is split ON HOST into xa = fp16(x) plus an fp8e4m3 residual
    stream xr = e4m3((x - xa) * 2^9), contracted against +-2^-9 weights
    (exactly representable e4m3 subnormals) with fp8 DoubleRow matmuls.
    7 fp16 matmuls + 4 DR matmuls per m-tile replace the 14 fp16
    matmuls of a two-stream fp16 split: ~2^-16.7 effective x precision,
    final rel err ~7.6e-3 (measured in f64 simulation) vs 2e-2 budget.
  - L2/L3: both operands are exactly +-1/0 in fp8e4 -> DoubleRow fp8
    matmuls produce bit-exact integer sums in fp32 PSUM.
  - L4 runs transposed (w4 [128,10] stationary, h3 moving) so the PE
    streams 256-col moving operands instead of 10-col ones; [10, b]
    logit tiles are PE-transposed back for the row-major epilogue.
  - BN + bias folding: bn(h + b) = A*h + C with A = g*rsqrt(v+eps),
    C = A*(b - m) + beta, applied per-partition by the Sign/Identity
    activations (fp32 internally).
"""

import sys

if "/opt/trn_rl_repo" not in sys.path:
    sys.path.insert(0, "/opt/trn_rl_repo")

import numpy as np

D_IN, H1, H2, H3, NCLS = 784, 3072, 1536, 768, 10
B, NCORES = 16384, 8
BC = B // NCORES          # batch rows per core
NB = 256                  # batch columns processed per chunk (L2-L4)
KP = 112                  # L1 fp16 k-tile partition size (784 = 7 * 112)
K1T = D_IN // KP          # 7
KR = 1024                 # L1 residual k rows (784 padded to 8 * 128)
KRT = KR // 128           # 8
KRP = KRT // 2            # 4 DoubleRow pair iterations
M1, M2, M3 = H1 // 128, H2 // 128, H3 // 128   # 24, 12, 6
K2P, K3P = H1 // 256, H2 // 256                # DoubleRow k-pair iters: 12, 6
K4T = H3 // 128                                # 6
BN_EPS = 1e-5
RSC = 512.0               # residual scale 2^9; weights are +-2^-9

_cached = {}


def _build(bc):
    import concourse.bacc as bacc
    import concourse.mybir as mybir
    import concourse.tile as tile

    dt = mybir.dt
    AF = mybir.ActivationFunctionType
    PM = mybir.MatmulPerfMode
    ALU = mybir.AluOpType

    assert bc % NB == 0 and NB % 128 == 0
    gbts = bc // 128  # output row-tiles per core

    nc = bacc.Bacc("TRN2", target_bir_lowering=False, debug=False,
                   num_devices=NCORES)

    # x / w1 streams are pre-rearranged on host so every DMA moves long
    # contiguous per-partition lines:
    #   xa16 [112, G, 7, W]   (g, kt)-sliced fp16 x
    #   xr8  [128, G, 8, W]   scaled e4m3 residual (k padded 784 -> 1024)
    #   w1t  [112, 7, 3072]   fp16 sign(w1).T, sliced in 1024-col chunks
    #   w1r  [128, 6, 8, 512] e4m3 +-2^-9 sign(w1).T in 512-col m-blocks
    NG = bc // (2 * NB) if bc % (2 * NB) == 0 else bc // NB
    WG = bc // NG
    xa16 = nc.declare_dram_parameter("xa16", [KP, NG, K1T, WG], dt.float16,
                                     isOutput=False)
    xr8 = nc.declare_dram_parameter("xr8", [128, NG, KRT, WG], dt.float8e4,
                                    isOutput=False)
    w1t = nc.declare_dram_parameter("w1t", [KP, K1T, H1], dt.float16,
                                    isOutput=False)
    w1r = nc.declare_dram_parameter("w1r", [128, M1 // 4, KRT, 512],
                                    dt.float8e4, isOutput=False)
    w2t = nc.declare_dram_parameter("w2t", [H1, H2], dt.float8e4, isOutput=False)
    w3t = nc.declare_dram_parameter("w3t", [H2, H3], dt.float8e4, isOutput=False)
    w4t = nc.declare_dram_parameter("w4t", [H3, NCLS], dt.bfloat16, isOutput=False)
    a1s = nc.declare_dram_parameter("a1s", [128, M1], dt.float32, isOutput=False)
    c1s = nc.declare_dram_parameter("c1s", [128, M1], dt.float32, isOutput=False)
    a2s = nc.declare_dram_parameter("a2s", [128, M2], dt.float32, isOutput=False)
    c2s = nc.declare_dram_parameter("c2s", [128, M2], dt.float32, isOutput=False)
    a3s = nc.declare_dram_parameter("a3s", [128, M3], dt.float32, isOutput=False)
    c3s = nc.declare_dram_parameter("c3s", [128, M3], dt.float32, isOutput=False)
    b4s = nc.declare_dram_parameter("b4s", [128, NCLS], dt.float32, isOutput=False)
    id10 = nc.declare_dram_parameter("id10", [NCLS, NCLS], dt.float32,
                                     isOutput=False)
    out = nc.declare_dram_parameter("out", [bc, NCLS], dt.float32, isOutput=True)

    with tile.TileContext(nc) as tc, \
            tc.tile_pool(name="wts", bufs=1) as wp, \
            tc.tile_pool(name="xin", bufs=2) as xp, \
            tc.tile_pool(name="act", bufs=2) as ap_, \
            tc.tile_pool(name="eps", bufs=2) as ep, \
            tc.tile_pool(name="ps", bufs=4, space="PSUM") as ps, \
            tc.tile_pool(name="psz", bufs=1, space="PSUM") as psz:

        # ---- startup-critical transfers first: consts, chunk-0 x streams,
        # first residual-weight block, then w1.  w2/w3 streams are
        # dependency-chained onto chunk-0 compute milestones below so they
        # don't steal HBM bandwidth at startup.
        a1sb = wp.tile([128, M1], dt.float32, tag="a1")
        c1sb = wp.tile([128, M1], dt.float32, tag="c1")
        a2sb = wp.tile([128, M2], dt.float32, tag="a2")
        c2sb = wp.tile([128, M2], dt.float32, tag="c2")
        a3sb = wp.tile([128, M3], dt.float32, tag="a3")
        c3sb = wp.tile([128, M3], dt.float32, tag="c3")
        b4sb = wp.tile([128, NCLS], dt.float32, tag="b4")
        id10sb = wp.tile([NCLS, NCLS], dt.float32, tag="id10")
        for sb, drh in ((a1sb, a1s), (c1sb, c1s), (a2sb, a2s), (c2sb, c2s),
                        (a3sb, a3s), (c3sb, c3s), (b4sb, b4s), (id10sb, id10)):
            nc.sync.dma_start(sb[:], drh[:])

        # L1 runs on wide batch groups (W columns); L2-L4 iterate over
        # NB-column halves of each group.
        W = 2 * NB if bc % (2 * NB) == 0 else NB
        ngroups = bc // W
        halves = W // NB
        NBLK = M1 // 4            # m-tile blocks of 4 (PSUM rotation depth)

        def load_x(g):
            # returns (list of per-k-tile fp16 APs, residual [128,KRT,W] AP,
            #          first dma)
            if g == 0:
                # group 0 is startup-latency critical: separate tiles per
                # k-tile so each matmul depends only on its own k-tile's DMA
                xas = []
                xdma = None
                for k in range(K1T):
                    xak = xp.tile([KP, W], dt.float16, tag=f"xa{k}", bufs=1,
                                  name=f"xa{k}")
                    d = nc.sync.dma_start(xak[:], xa16.ap()[:, 0, k, :])
                    xdma = xdma or d
                    xas.append(xak)
                xrg = xp.tile([128, KRT, W], dt.float8e4, tag="xr0", bufs=1,
                              name="xr0")
                nc.sync.dma_start(xrg[:], xr8.ap()[:, 0, :, :])
                return xas, xrg, xdma
            xag = xp.tile([KP, K1T, W], dt.float16, tag="xa")
            xdma = nc.sync.dma_start(xag[:], xa16.ap()[:, g, :, :])
            xrg = xp.tile([128, KRT, W], dt.float8e4, tag="xr")
            xrdma = nc.sync.dma_start(xrg[:], xr8.ap()[:, g, :, :])
            return ([xag[:, k, :] for k in range(K1T)], xrg, xdma)

        x0 = load_x(0)

        # w1 streams in 1024-col chunks so group-0 m-tile blocks are fed
        # as they compute; chunk 0 + the first two residual blocks go out
        # up-front, later chunks are staged behind group-0 L1 progress.
        MC = 3                    # 1024-col w1 chunks
        staged = {}               # m-tile act -> dmas released by it
        w1c = [[None] * MC for _ in range(K1T)]
        w1rb = [None] * NBLK

        def _w1_chunk(mc):
            dmas = []
            for kt in range(K1T):
                w1c[kt][mc] = wp.tile([KP, 1024], dt.float16,
                                      tag=f"w1_{kt}_{mc}",
                                      name=f"w1_{kt}_{mc}")
                dmas.append(nc.sync.dma_start(
                    w1c[kt][mc][:],
                    w1t.ap()[:, kt, mc * 1024:(mc + 1) * 1024]))
            return dmas

        def _w1r_block(mg):
            w1rb[mg] = wp.tile([128, KRT, 512], dt.float8e4, tag=f"w1r{mg}",
                               name=f"w1r{mg}")
            return nc.sync.dma_start(w1rb[mg][:], w1r.ap()[:, mg, :, :])

        _w1_chunk(0)
        _w1r_block(0)
        _w1r_block(1)
        staged[0] = _w1_chunk(1) + [_w1r_block(2), _w1r_block(3)]
        staged[8] = _w1_chunk(2) + [_w1r_block(4), _w1r_block(5)]

        w4sb = wp.tile([128, K4T, NCLS], dt.bfloat16, tag="w4")
        nc.sync.dma_start(w4sb[:], w4t.ap().rearrange("(kt p) n -> p kt n", p=128))

        w2sb = wp.tile([128, 2 * K2P, H2], dt.float8e4, tag="w2")
        w2_dmas = [
            nc.sync.dma_start(w2sb[:, kt, :], w2t[kt * 128:(kt + 1) * 128, :])
            for kt in range(2 * K2P)
        ]
        w3sb = wp.tile([128, 2 * K3P, H3], dt.float8e4, tag="w3")
        w3_dmas = [
            nc.sync.dma_start(w3sb[:, kt, :], w3t[kt * 128:(kt + 1) * 128, :])
            for kt in range(2 * K3P)
        ]

        zout = wp.tile([128, gbts, NCLS], dt.float32, tag="zout")
        ssum = wp.tile([128, gbts], dt.float32, tag="ssum")
        lsum = wp.tile([128, gbts], dt.float32, tag="lsum")

        def emit_epilogue(lo, hi):
            # log_softmax over the free dim; |z| is small so no max-shift
            for g in range(lo, hi):
                e = ep.tile([128, NCLS], dt.float32, tag="e")
                nc.scalar.activation(e[:], zout[:, g, :], AF.Exp,
                                     accum_out=ssum[:, g:g + 1])
            nc.scalar.activation(lsum[:, lo:hi], ssum[:, lo:hi], AF.Ln)
            for g in range(lo, hi):
                nc.vector.tensor_scalar(zout[:, g, :], zout[:, g, :],
                                        lsum[:, g:g + 1], None,
                                        op0=ALU.subtract)
            nc.sync.dma_start(
                out.ap()[lo * 128:hi * 128, :].rearrange("(g p) n -> p g n",
                                                         p=128),
                zout[:, lo:hi, :])

        prev_act0 = None
        act_g0_late = None
        for g in range(ngroups):
            xa, xrg, _ = x0 if g == 0 else (None, None, None)
            if g != 0:
                xa, xrg, xdma = load_x(g)
                # keep ~one group of x lookahead; group 1's prefetch waits
                # for late group-0 progress so it doesn't fight the
                # startup weight transfers
                gate = act_g0_late if g == 1 else prev_act0
                if gate is not None:
                    tile.add_dep_helper(xdma.ins, gate.ins, sync=True,
                                        reason="x prefetch staging")

            # ---- L1: [784 -> 3072] = 7 fp16 matmuls + 4 fp8 DoubleRow
            # residual matmuls per m-tile, blocks of 4 m-tiles over 4 PSUM
            # banks, kt-outer so the PE consumes each w1/x k-tile as its
            # DMA lands at startup.
            h1sb = ap_.tile([128, 2 * K2P, W], dt.float8e4, tag="h1")

            def l1_sign(mt, pt):
                act = nc.scalar.activation(h1sb[:, mt, :], pt[:], AF.Sign,
                                           bias=c1sb[:, mt:mt + 1],
                                           scale=a1sb[:, mt:mt + 1])
                if g == 0:
                    # stage w1-chunk/w2/w3 streams behind group-0 L1
                    # progress so they don't starve the startup transfers
                    for wd in staged.get(mt, ()):
                        tile.add_dep_helper(wd.ins, act.ins, sync=True,
                                            reason="w1 stream staging")
                    for wd_list, base in ((w2_dmas, 0), (w3_dmas, M1 // 2)):
                        for kt2, wd in enumerate(wd_list):
                            if base + kt2 // 2 == mt:
                                tile.add_dep_helper(
                                    wd.ins, act.ins, sync=True,
                                    reason="weight stream staging")
                return act

            for mg in range(NBLK):
                pts = [ps.tile([128, W], dt.float32, tag="ps",
                               name=f"pt{i}") for i in range(4)]
                for kt in range(K1T):
                    for i in range(4):
                        mt = mg * 4 + i
                        lhs = w1c[kt][mt // 8][:, (mt % 8) * 128:
                                               (mt % 8 + 1) * 128]
                        nc.tensor.matmul(pts[i][:], lhs, xa[kt][:],
                                         start=(kt == 0), stop=False)
                for i in range(4):
                    mt = mg * 4 + i
                    for rp in range(KRP):
                        nc.tensor.matmul(
                            pts[i][:],
                            w1rb[mg][:, 2 * rp:2 * rp + 2,
                                     i * 128:(i + 1) * 128],
                            xrg[:, 2 * rp:2 * rp + 2, :],
                            start=False, stop=(rp == KRP - 1),
                            perf_mode=PM.DoubleRow)
                    a = l1_sign(mt, pts[i])
                    if mt == 0:
                        prev_act0 = a
                    if g == 0 and mt == 20:
                        act_g0_late = a

            for h in range(halves):
                hs = slice(h * NB, (h + 1) * NB)
                # ---- L2: [3072 -> 1536], fp8 DoubleRow
                h2sb = ap_.tile([128, 2 * K3P, NB], dt.float8e4, tag="h2")
                for mt in range(M2):
                    pt = ps.tile([128, NB], dt.float32, tag="ps")
                    for kp in range(K2P):
                        nc.tensor.matmul(
                            pt[:],
                            w2sb[:, 2 * kp:2 * kp + 2, mt * 128:(mt + 1) * 128],
                            h1sb[:, 2 * kp:2 * kp + 2, hs],
                            start=(kp == 0), stop=(kp == K2P - 1),
                            perf_mode=PM.DoubleRow)
                    nc.scalar.activation(h2sb[:, mt, :], pt[:], AF.Sign,
                                         bias=c2sb[:, mt:mt + 1],
                                         scale=a2sb[:, mt:mt + 1])

                # ---- L3: [1536 -> 768], fp8 DoubleRow; output clipped bf16
                h3c = ap_.tile([128, K4T, NB], dt.bfloat16, tag="h3")
                for mt in range(M3):
                    pt = ps.tile([128, NB], dt.float32, tag="ps")
                    for kp in range(K3P):
                        nc.tensor.matmul(
                            pt[:],
                            w3sb[:, 2 * kp:2 * kp + 2, mt * 128:(mt + 1) * 128],
                            h2sb[:, 2 * kp:2 * kp + 2, :],
                            start=(kp == 0), stop=(kp == K3P - 1),
                            perf_mode=PM.DoubleRow)
                    nc.vector.tensor_scalar(h3c[:, mt, :], pt[:],
                                            a3sb[:, mt:mt + 1],
                                            c3sb[:, mt:mt + 1],
                                            op0=ALU.mult, op1=ALU.add)
                    nc.vector.tensor_scalar(h3c[:, mt, :], h3c[:, mt, :],
                                            1.0, -1.0, op0=ALU.min,
                                            op1=ALU.max)

                # ---- L4: z.T = w4 @ y3.T, w4 stationary [128,10], h3
                # moving [128,NB]; then PE-transpose [10,128] chunks back
                # to row-major and add b4.
                zp = psz.tile([NCLS, NB], dt.float32, tag="zp")
                for kt in range(K4T):
                    nc.tensor.matmul(zp[:], w4sb[:, kt, :], h3c[:, kt, :],
                                     start=(kt == 0), stop=(kt == K4T - 1))
                zs = ep.tile([NCLS, NB], dt.float32, tag="zs")
                nc.vector.tensor_copy(zs[:], zp[:])
                for bt in range(NB // 128):
                    gbt = (g * halves + h) * (NB // 128) + bt
                    ztp = psz.tile([128, NCLS], dt.float32, tag="zt")
                    nc.tensor.transpose(ztp[:],
                                        zs[:, bt * 128:(bt + 1) * 128],
                                        id10sb[:])
                    nc.vector.tensor_add(zout[:, gbt, :], ztp[:], b4sb[:])

                if (g == ngroups - 1 and ngroups >= 2 and halves == 2
                        and h == 0):
                    # first half of the last group: epilogue overlaps the
                    # second half's matmuls
                    emit_epilogue(gbts - 4, gbts - 2)

            if g == ngroups - 2:
                # bulk of the log-softmax epilogue hides under the last
                # group's matmuls; only the final row-tiles run in the tail
                emit_epilogue(0, (g + 1) * W // 128)

        if ngroups >= 2 and halves == 2:
            emit_epilogue(gbts - 2, gbts)
        elif ngroups >= 2:
            emit_epilogue((ngroups - 1) * W // 128, gbts)
        else:
            emit_epilogue(0, gbts)

    nc.finalize()
    return nc


def _prep(x, w1, b1, w2, b2, w3, b3, w4, b4,
          g1, be1, m1, v1, g2, be2, m2, v2, g3, be3, m3, v3):
    """Host-side layout prep: transposes, binarized weight casts, BN folds,
    and the fp16 + scaled-e4m3-residual split of x."""
    import concourse.mybir as mybir
    f8 = mybir.dt.np(mybir.dt.float8e4)

    def fold(g, be, m, v, b):
        a = (g / np.sqrt(v + np.float32(BN_EPS))).astype(np.float32)
        c = (a * (b - m) + be).astype(np.float32)
        return a, c

    a1, c1 = fold(g1, be1, m1, v1, b1)
    a2, c2 = fold(g2, be2, m2, v2, b2)
    a3, c3 = fold(g3, be3, m3, v3, b3)

    def cols(v, mtiles):
        return np.ascontiguousarray(v.reshape(mtiles, 128).T)

    s1 = np.sign(w1).T  # [784, 3072]
    w1rp = np.zeros((KR, H1), dtype=f8)
    w1rp[:D_IN] = (s1 / RSC).astype(f8)
    # [1024, 3072] -> [128, 6 m-blocks, 8 k-tiles, 512]
    w1r4 = np.ascontiguousarray(
        w1rp.reshape(KRT, 128, M1 // 4, 512).transpose(1, 2, 0, 3))
    # [784, 3072] -> [112, 7 k-tiles, 3072]
    w1t3 = np.ascontiguousarray(
        s1.astype(np.float16).reshape(K1T, KP, H1).transpose(1, 0, 2))

    pre = dict(
        w1t=w1t3,
        w1r=w1r4,
        w2t=np.ascontiguousarray(np.sign(w2).T).astype(f8),
        w3t=np.ascontiguousarray(np.sign(w3).T).astype(f8),
        w4t=np.ascontiguousarray(w4.T).astype(mybir.dt.np(mybir.dt.bfloat16)),
        a1s=cols(a1, M1), c1s=cols(c1, M1),
        a2s=cols(a2, M2), c2s=cols(c2, M2),
        a3s=cols(a3, M3), c3s=cols(c3, M3),
        b4s=np.ascontiguousarray(np.tile(b4.astype(np.float32), (128, 1))),
        id10=np.eye(NCLS, dtype=np.float32),
    )
    xt = np.ascontiguousarray(x.T.astype(np.float32))  # [784, B]
    xa = xt.astype(np.float16)
    xr = np.zeros((KR, x.shape[0]), dtype=f8)
    xr[:D_IN] = ((xt - xa.astype(np.float32)) * np.float32(RSC)).astype(f8)
    return pre, xa, xr


def run(inputs, **spmd_kwargs):
    from concourse.bass_utils import run_bass_kernel_spmd

    if "nc" not in _cached:
        _cached["nc"] = _build(BC)
    nc = _cached["nc"]

    inputs = {k: np.asarray(v) for k, v in inputs.items()}
    pre, xa, xr = _prep(**inputs)

    W = 2 * NB if BC % (2 * NB) == 0 else NB
    NG = BC // W
    in_maps = []
    for core in range(NCORES):
        m = dict(pre)
        xac = xa[:, core * BC:(core + 1) * BC]
        xrc = xr[:, core * BC:(core + 1) * BC]
        # [784, bc] -> [112, G, 7, W]; [1024, bc] -> [128, G, 8, W]
        m["xa16"] = np.ascontiguousarray(
            xac.reshape(K1T, KP, NG, W).transpose(1, 2, 0, 3))
        m["xr8"] = np.ascontiguousarray(
            xrc.reshape(KRT, 128, NG, W).transpose(1, 2, 0, 3))
        in_maps.append(m)

    res = run_bass_kernel_spmd(nc, in_maps, list(range(NCORES)), **spmd_kwargs)
    outs = [res.results[i]["out"] for i in range(NCORES)]
    return res, np.concatenate(outs, axis=0).astype(np.float32)


def kernel(**inputs):
    return run(inputs)[1]


# revision 24
# speedup vs baseline: 1.3240x; 1.0798x over previous
"""Trainium2 Bass kernel for a binarized 4-layer MLP (eval mode).

Reference computation (per row of x [B=16384, 784]):
  h1 = x @ sign(w1).T + b1;  s1 = sign(bn1(h1))        (clip doesn't change sign)
  h2 = s1 @ sign(w2).T + b2; s2 = sign(bn2(h2))
  h3 = s2 @ sign(w3).T + b3; y3 = clip(bn3(h3), -1, 1)
  z  = y3 @ w4.T + b4;       out = log_softmax(z)

Sharding: pure data-parallel over the batch across 8 NeuronCores
(weights replicated, no collectives).

Numerics:
  - L1: x is split ON HOST into xa = fp16(x) plus an fp8e4m3 residual
    stream xr = e4m3((x - xa) * 2^9), contracted against +-2^-9 weights
    (exactly representable e4m3 subnormals) with fp8 DoubleRow matmuls.
    7 fp16 matmuls + 4 DR matmuls per m-tile replace the 14 fp16
    matmuls of a two-stream fp16 split: ~2^-16.7 effective x precision,
    final rel err ~7.6e-3 (measured in f64 simulation) vs 2e-2 budget.
  - L2/L3: both operands are exactly +-1/0 in fp8e4 -> DoubleRow fp8
    matmuls produce bit-exact integer sums in fp32 PSUM.
  - L4 runs transposed (w4 [128,10] stationary, h3 moving) so the PE
    streams 256-col moving operands instead of 10-col ones; [10, b]
    logit tiles are PE-transposed back for the row-major epilogue.
  - BN + bias folding: bn(h + b) = A*h + C with A = g*rsqrt(v+eps),
    C = A*(b - m) + beta, applied per-partition by the Sign/Identity
    activations (fp32 internally).
"""

import sys

if "/opt/trn_rl_repo" not in sys.path:
    sys.path.insert(0, "/opt/trn_rl_repo")

import numpy as np

D_IN, H1, H2, H3, NCLS = 784, 3072, 1536, 768, 10
B, NCORES = 16384, 8
BC = B // NCORES          # batch rows per core
NB = 256                  # batch columns processed per chunk (L2-L4)
KF = 768                  # rows 0-767 go through the fp16 stream
K1T = KF // 128           # 6 full 128-row fp16 k-tiles
KR = 1024                 # L1 fp8 stream k rows (residuals + row chunks)
KRT = KR // 128           # 8
KRP = KRT // 2            # 4 DoubleRow pair iterations
M1, M2, M3 = H1 // 128, H2 // 128, H3 // 128   # 24, 12, 6
K2P, K3P = H1 // 256, H2 // 256                # DoubleRow k-pair iters: 12, 6
K4T = H3 // 128                                # 6
BN_EPS = 1e-5
RSC = 512.0               # fp8-stream scale 2^9; weights are +-2^-9
# const-block column layout: a1 c1 a2 c2 a3 c3 b4 id10
_C0 = [0, M1, 2 * M1, 2 * M1 + M2, 2 * (M1 + M2), 2 * (M1 + M2) + M3,
       2 * (M1 + M2 + M3), 2 * (M1 + M2 + M3) + NCLS]
NCONST = 2 * (M1 + M2 + M3) + 2 * NCLS

_cached = {}


def _build(bc):
    import concourse.bacc as bacc
    import concourse.mybir as mybir
    import concourse.tile as tile

    dt = mybir.dt
    AF = mybir.ActivationFunctionType
    PM = mybir.MatmulPerfMode
    ALU = mybir.AluOpType

    assert bc % NB == 0 and NB % 128 == 0
    gbts = bc // 128  # output row-tiles per core

    nc = bacc.Bacc("TRN2", target_bir_lowering=False, debug=False,
                   num_devices=NCORES)

    # x / w1 streams are pre-rearranged on host so every DMA moves long
    # contiguous per-partition lines:
    #   xa16 [128, G, 6, W]   (g, kt)-sliced fp16 x rows 0-767
    #   xr8  [128, G, 8, W]   e4m3 stream: scaled residuals of rows 0-767
    #                         plus 4x 4-bit chunks of rows 768-783
    #   w1t  [128, 6, 3072]   fp16 sign(w1).T rows 0-767, 1024-col chunks
    #   w1r  [128, 6, 8, 512] e4m3 fp8-stream weights in 512-col m-blocks
    NG = bc // (2 * NB) if bc % (2 * NB) == 0 else bc // NB
    WG = bc // NG
    xa16 = nc.declare_dram_parameter("xa16", [128, NG, K1T, WG], dt.float16,
                                     isOutput=False)
    xr8 = nc.declare_dram_parameter("xr8", [128, NG, KRT, WG], dt.float8e4,
                                    isOutput=False)
    w1t = nc.declare_dram_parameter("w1t", [128, K1T, H1], dt.float16,
                                    isOutput=False)
    w1r = nc.declare_dram_parameter("w1r", [128, M1 // 4, KRT, 512],
                                    dt.float8e4, isOutput=False)
    w2t = nc.declare_dram_parameter("w2t", [H1, H2], dt.float8e4, isOutput=False)
    w3t = nc.declare_dram_parameter("w3t", [H2, H3], dt.float8e4, isOutput=False)
    w4t = nc.declare_dram_parameter("w4t", [H3, NCLS], dt.bfloat16, isOutput=False)
    csts = nc.declare_dram_parameter("csts", [128, NCONST], dt.float32,
                                     isOutput=False)
    out = nc.declare_dram_parameter("out", [bc, NCLS], dt.float32, isOutput=True)

    with tile.TileContext(nc) as tc, \
            tc.tile_pool(name="wts", bufs=1) as wp, \
            tc.tile_pool(name="xin", bufs=2) as xp, \
            tc.tile_pool(name="act", bufs=2) as ap_, \
            tc.tile_pool(name="eps", bufs=2) as ep, \
            tc.tile_pool(name="ps", bufs=4, space="PSUM") as ps, \
            tc.tile_pool(name="psz", bufs=1, space="PSUM") as psz:

        # ---- startup-critical transfers first: chunk-0 x streams and the
        # first w1 chunk go out ahead of everything (small DMAs have ~1.4us
        # fixed latency each, so the consts ride in ONE packed transfer,
        # emitted after the compute-critical streams).  w2/w3 streams are
        # dependency-chained onto chunk-0 compute milestones below so they
        # don't steal HBM bandwidth at startup.

        # L1 runs on wide batch groups (W columns); L2-L4 iterate over
        # NB-column halves of each group.
        W = 2 * NB if bc % (2 * NB) == 0 else NB
        ngroups = bc // W
        halves = W // NB
        NBLK = M1 // 4            # m-tile blocks of 4 (PSUM rotation depth)

        def load_x(g):
            # returns (list of per-k-tile fp16 APs, residual [128,KRT,W] AP,
            #          first dma)
            if g == 0:
                # group 0 is startup-latency critical: separate tiles per
                # k-tile so each matmul depends only on its own k-tile's
                # DMA.  The very first matmul needs xa[0] + w1 chunk-0
                # kt=0, so those two transfers are emitted before anything
                # else (see the w1 chunk emission below).
                xas = []
                xdma = None
                for k in range(K1T):
                    xak = xp.tile([128, W], dt.float16, tag=f"xa{k}", bufs=1,
                                  name=f"xa{k}")
                    d = nc.sync.dma_start(xak[:], xa16.ap()[:, 0, k, :])
                    xdma = xdma or d
                    xas.append(xak)
                    if k == 0:
                        # w1 chunk-0 k-tile 0 right behind xa[0]
                        _w1_chunk_kt(0, 0)
                xrg = xp.tile([128, KRT, W], dt.float8e4, tag="xr0", bufs=1,
                              name="xr0")
                nc.sync.dma_start(xrg[:], xr8.ap()[:, 0, :, :])
                return xas, xrg, xdma
            xag = xp.tile([128, K1T, W], dt.float16, tag="xa")
            xdma = nc.sync.dma_start(xag[:], xa16.ap()[:, g, :, :])
            xrg = xp.tile([128, KRT, W], dt.float8e4, tag="xr")
            xrdma = nc.sync.dma_start(xrg[:], xr8.ap()[:, g, :, :])
            return ([xag[:, k, :] for k in range(K1T)], xrg, xdma)

        # w1 streams in 1024-col chunks so group-0 m-tile blocks are fed
        # as they compute; chunk 0 + the first two residual blocks go out
        # up-front, later chunks are staged behind group-0 L1 progress.
        MC = 3                    # 1024-col w1 chunks
        staged = {}               # m-tile act -> dmas released by it
        w1c = [[None] * MC for _ in range(K1T)]
        w1rb = [None] * NBLK

        def _w1_chunk_kt(kt, mc):
            w1c[kt][mc] = wp.tile([128, 1024], dt.float16,
                                  tag=f"w1_{kt}_{mc}",
                                  name=f"w1_{kt}_{mc}")
            return nc.sync.dma_start(
                w1c[kt][mc][:],
                w1t.ap()[:, kt, mc * 1024:(mc + 1) * 1024])

        def _w1_chunk(mc):
            return [_w1_chunk_kt(kt, mc) for kt in range(K1T)
                    if w1c[kt][mc] is None]

        def _w1r_block(mg):
            w1rb[mg] = wp.tile([128, KRT, 512], dt.float8e4, tag=f"w1r{mg}",
                               name=f"w1r{mg}")
            return nc.sync.dma_start(w1rb[mg][:], w1r.ap()[:, mg, :, :])

        x0 = load_x(0)
        _w1_chunk(0)
        _w1r_block(0)
        _w1r_block(1)

        cstb = wp.tile([128, NCONST], dt.float32, tag="csts")
        nc.sync.dma_start(cstb[:], csts[:])
        a1sb = cstb[:, _C0[0]:_C0[0] + M1]
        c1sb = cstb[:, _C0[1]:_C0[1] + M1]
        a2sb = cstb[:, _C0[2]:_C0[2] + M2]
        c2sb = cstb[:, _C0[3]:_C0[3] + M2]
        a3sb = cstb[:, _C0[4]:_C0[4] + M3]
        c3sb = cstb[:, _C0[5]:_C0[5] + M3]
        b4sb = cstb[:, _C0[6]:_C0[6] + NCLS]
        id10sb = cstb[0:NCLS, _C0[7]:_C0[7] + NCLS]

        w4sb = wp.tile([128, K4T, NCLS], dt.bfloat16, tag="w4")
        nc.sync.dma_start(w4sb[:], w4t.ap().rearrange("(kt p) n -> p kt n", p=128))

        staged[0] = _w1_chunk(1) + [_w1r_block(2), _w1r_block(3)]
        staged[8] = _w1_chunk(2) + [_w1r_block(4), _w1r_block(5)]

        w2sb = wp.tile([128, 2 * K2P, H2], dt.float8e4, tag="w2")
        w2_dmas = [
            nc.sync.dma_start(w2sb[:, kt, :], w2t[kt * 128:(kt + 1) * 128, :])
            for kt in range(2 * K2P)
        ]
        w3sb = wp.tile([128, 2 * K3P, H3], dt.float8e4, tag="w3")
        w3_dmas = [
            nc.sync.dma_start(w3sb[:, kt, :], w3t[kt * 128:(kt + 1) * 128, :])
            for kt in range(2 * K3P)
        ]

        zout = wp.tile([128, gbts, NCLS], dt.float32, tag="zout")
        ssum = wp.tile([128, gbts], dt.float32, tag="ssum")
        lsum = wp.tile([128, gbts], dt.float32, tag="lsum")

        def emit_epilogue(lo, hi):
            # log_softmax over the free dim; |z| is small so no max-shift
            for g in range(lo, hi):
                e = ep.tile([128, NCLS], dt.float32, tag="e")
                nc.scalar.activation(e[:], zout[:, g, :], AF.Exp,
                                     accum_out=ssum[:, g:g + 1])
            nc.scalar.activation(lsum[:, lo:hi], ssum[:, lo:hi], AF.Ln)
            for g in range(lo, hi):
                nc.vector.tensor_scalar(zout[:, g, :], zout[:, g, :],
                                        lsum[:, g:g + 1], None,
                                        op0=ALU.subtract)
            nc.sync.dma_start(
                out.ap()[lo * 128:hi * 128, :].rearrange("(g p) n -> p g n",
                                                         p=128),
                zout[:, lo:hi, :])

        prev_act0 = None
        act_g0_late = None
        for g in range(ngroups):
            xa, xrg, _ = x0 if g == 0 else (None, None, None)
            if g != 0:
                xa, xrg, xdma = load_x(g)
                # keep ~one group of x lookahead; group 1's prefetch waits
                # for late group-0 progress so it doesn't fight the
                # startup weight transfers
                gate = act_g0_late if g == 1 else prev_act0
                if gate is not None:
                    tile.add_dep_helper(xdma.ins, gate.ins, sync=True,
                                        reason="x prefetch staging")

            # ---- L1: [784 -> 3072] = 7 fp16 matmuls + 4 fp8 DoubleRow
            # residual matmuls per m-tile, blocks of 4 m-tiles over 4 PSUM
            # banks, kt-outer so the PE consumes each w1/x k-tile as its
            # DMA lands at startup.
            h1sb = ap_.tile([128, 2 * K2P, W], dt.float8e4, tag="h1")

            def l1_sign(mt, pt):
                act = nc.scalar.activation(h1sb[:, mt, :], pt[:], AF.Sign,
                                           bias=c1sb[:, mt:mt + 1],
                                           scale=a1sb[:, mt:mt + 1])
                if g == 0:
                    # stage w1-chunk/w2/w3 streams behind group-0 L1
                    # progress so they don't starve the startup transfers
                    for wd in staged.get(mt, ()):
                        tile.add_dep_helper(wd.ins, act.ins, sync=True,
                                            reason="w1 stream staging")
                    for wd_list, base in ((w2_dmas, 0), (w3_dmas, M1 // 2)):
                        for kt2, wd in enumerate(wd_list):
                            if base + kt2 // 2 == mt:
                                tile.add_dep_helper(
                                    wd.ins, act.ins, sync=True,
                                    reason="weight stream staging")
                return act

            for mg in range(NBLK):
                pts = [ps.tile([128, W], dt.float32, tag="ps",
                               name=f"pt{i}") for i in range(4)]
                for kt in range(K1T):
                    for i in range(4):
                        mt = mg * 4 + i
                        lhs = w1c[kt][mt // 8][:, (mt % 8) * 128:
                                               (mt % 8 + 1) * 128]
                        nc.tensor.matmul(pts[i][:], lhs, xa[kt][:],
                                         start=(kt == 0), stop=False)
                for i in range(4):
                    mt = mg * 4 + i
                    for rp in range(KRP):
                        nc.tensor.matmul(
                            pts[i][:],
                            w1rb[mg][:, 2 * rp:2 * rp + 2,
                                     i * 128:(i + 1) * 128],
                            xrg[:, 2 * rp:2 * rp + 2, :],
                            start=False, stop=(rp == KRP - 1),
                            perf_mode=PM.DoubleRow)
                    a = l1_sign(mt, pts[i])
                    if mt == 0:
                        prev_act0 = a
                    if g == 0 and mt == 20:
                        act_g0_late = a

            # all groups but the last run L2-L4 at the full W columns
            # (fewer instructions); the last group runs NB-column halves so
            # the serial L3->L4->epilogue tail telescopes on a small piece.
            HB = NB if g == ngroups - 1 else W
            for h in range(W // HB):
                hs = slice(h * HB, (h + 1) * HB)
                # ---- L2: [3072 -> 1536], fp8 DoubleRow
                h2sb = ap_.tile([128, 2 * K3P, HB], dt.float8e4,
                                tag=f"h2_{HB}")
                for mt in range(M2):
                    pt = ps.tile([128, HB], dt.float32, tag="ps")
                    for kp in range(K2P):
                        nc.tensor.matmul(
                            pt[:],
                            w2sb[:, 2 * kp:2 * kp + 2, mt * 128:(mt + 1) * 128],
                            h1sb[:, 2 * kp:2 * kp + 2, hs],
                            start=(kp == 0), stop=(kp == K2P - 1),
                            perf_mode=PM.DoubleRow)
                    nc.scalar.activation(h2sb[:, mt, :], pt[:], AF.Sign,
                                         bias=c2sb[:, mt:mt + 1],
                                         scale=a2sb[:, mt:mt + 1])

                # ---- L3: [1536 -> 768], fp8 DoubleRow; output clipped bf16
                h3c = ap_.tile([128, K4T, HB], dt.bfloat16, tag=f"h3_{HB}")
                for mt in range(M3):
                    pt = ps.tile([128, HB], dt.float32, tag="ps")
                    for kp in range(K3P):
                        nc.tensor.matmul(
                            pt[:],
                            w3sb[:, 2 * kp:2 * kp + 2, mt * 128:(mt + 1) * 128],
                            h2sb[:, 2 * kp:2 * kp + 2, :],
                            start=(kp == 0), stop=(kp == K3P - 1),
                            perf_mode=PM.DoubleRow)
                    nc.vector.tensor_scalar(h3c[:, mt, :], pt[:],
                                            a3sb[:, mt:mt + 1],
                                            c3sb[:, mt:mt + 1],
                                            op0=ALU.mult, op1=ALU.add)
                    nc.vector.tensor_scalar(h3c[:, mt, :], h3c[:, mt, :],
                                            1.0, -1.0, op0=ALU.min,
                                            op1=ALU.max)

                # ---- L4: z.T = w4 @ y3.T, w4 stationary [128,10], h3
                # moving [128,HB]; then PE-transpose [10,128] chunks back
                # to row-major and add b4.
                zp = psz.tile([NCLS, W], dt.float32, tag="zp")
                for kt in range(K4T):
                    nc.tensor.matmul(zp[:, hs], w4sb[:, kt, :], h3c[:, kt, :],
                                     start=(kt == 0), stop=(kt == K4T - 1))
                zs = ep.tile([NCLS, HB], dt.float32, tag=f"zs_{HB}")
                nc.vector.tensor_copy(zs[:], zp[:, hs])
                for bt in range(HB // 128):
                    gbt = (g * W + h * HB) // 128 + bt
                    ztp = psz.tile([128, NCLS], dt.float32, tag="zt")
                    nc.tensor.transpose(ztp[:],
                                        zs[:, bt * 128:(bt + 1) * 128],
                                        id10sb)
                    nc.vector.tensor_add(zout[:, gbt, :], ztp[:], b4sb)

                if g == ngroups - 1 and ngroups >= 2 and HB < W and h == 0:
                    # first half of the last group: epilogue overlaps the
                    # second half's matmuls
                    emit_epilogue(gbts - 4, gbts - 2)

            if g == ngroups - 2:
                # bulk of the log-softmax epilogue hides under the last
                # group's matmuls; only the final row-tiles run in the tail
                emit_epilogue(0, (g + 1) * W // 128)

        if ngroups >= 2 and halves == 2:
            # per-row-tile pieces so each DMA overlaps the next tile's chain
            emit_epilogue(gbts - 2, gbts - 1)
            emit_epilogue(gbts - 1, gbts)
        elif ngroups >= 2:
            emit_epilogue((ngroups - 1) * W // 128, gbts)
        else:
            emit_epilogue(0, gbts)

    nc.finalize()
    return nc


def _prep(x, w1, b1, w2, b2, w3, b3, w4, b4,
          g1, be1, m1, v1, g2, be2, m2, v2, g3, be3, m3, v3):
    """Host-side layout prep: transposes, binarized weight casts, BN folds,
    and the fp16 + scaled-e4m3-residual split of x."""
    import concourse.mybir as mybir
    f8 = mybir.dt.np(mybir.dt.float8e4)

    def fold(g, be, m, v, b):
        a = (g / np.sqrt(v + np.float32(BN_EPS))).astype(np.float32)
        c = (a * (b - m) + be).astype(np.float32)
        return a, c

    a1, c1 = fold(g1, be1, m1, v1, b1)
    a2, c2 = fold(g2, be2, m2, v2, b2)
    a3, c3 = fold(g3, be3, m3, v3, b3)

    def cols(v, mtiles):
        return np.ascontiguousarray(v.reshape(mtiles, 128).T)

    s1 = np.sign(w1).T.astype(np.float32)  # [784, 3072]
    NX = D_IN - KF                         # 16 leftover rows
    # fp8-stream weight rows: residual weights for rows 0-767, then the
    # leftover-row chunk weights (+-1 for chunk 0, +-2^-9 for chunks 1-3)
    w1rp = np.zeros((KR, H1), dtype=f8)
    w1rp[:KF] = (s1[:KF] / np.float32(RSC)).astype(f8)
    w1rp[KF:KF + NX] = s1[KF:].astype(f8)
    for j in range(1, 4):
        w1rp[KF + j * NX:KF + (j + 1) * NX] = (s1[KF:] / np.float32(RSC)).astype(f8)
    # [1024, 3072] -> [128, 6 m-blocks, 8 k-tiles, 512]
    w1r4 = np.ascontiguousarray(
        w1rp.reshape(KRT, 128, M1 // 4, 512).transpose(1, 2, 0, 3))
    # [768, 3072] -> [128, 6 k-tiles, 3072]
    w1t3 = np.ascontiguousarray(
        s1[:KF].astype(np.float16).reshape(K1T, 128, H1).transpose(1, 0, 2))

    cst = np.zeros((128, NCONST), dtype=np.float32)
    for i, v in enumerate((cols(a1, M1), cols(c1, M1), cols(a2, M2),
                           cols(c2, M2), cols(a3, M3), cols(c3, M3),
                           np.tile(b4.astype(np.float32), (128, 1)))):
        cst[:, _C0[i]:_C0[i] + v.shape[1]] = v
    cst[:NCLS, _C0[7]:_C0[7] + NCLS] = np.eye(NCLS, dtype=np.float32)

    pre = dict(
        w1t=w1t3,
        w1r=w1r4,
        w2t=np.ascontiguousarray(np.sign(w2).T).astype(f8),
        w3t=np.ascontiguousarray(np.sign(w3).T).astype(f8),
        w4t=np.ascontiguousarray(w4.T).astype(mybir.dt.np(mybir.dt.bfloat16)),
        csts=cst,
    )
    xt = np.ascontiguousarray(x.T.astype(np.float32))  # [784, B]
    xa = xt[:KF].astype(np.float16)
    xr = np.zeros((KR, x.shape[0]), dtype=f8)
    xr[:KF] = ((xt[:KF] - xa.astype(np.float32)) * np.float32(RSC)).astype(f8)
    # leftover rows 768-783: chain of four 4-bit e4m3 chunks
    r = xt[KF:]
    for j in range(4):
        s = np.float32(1.0 if j == 0 else RSC)
        q = (r * s).astype(f8)
        xr[KF + j * NX:KF + (j + 1) * NX] = q
        r = r - q.astype(np.float32) / s
    return pre, xa, xr


def run(inputs, **spmd_kwargs):
    from concourse.bass_utils import run_bass_kernel_spmd

    if "nc" not in _cached:
        _cached["nc"] = _build(BC)
    nc = _cached["nc"]

    inputs = {k: np.asarray(v) for k, v in inputs.items()}
    pre, xa, xr = _prep(**inputs)

    W = 2 * NB if BC % (2 * NB) == 0 else NB
    NG = BC // W
    in_maps = []
    for core in range(NCORES):
        m = dict(pre)
        xac = xa[:, core * BC:(core + 1) * BC]
        xrc = xr[:, core * BC:(core + 1) * BC]
        # [768, bc] -> [128, G, 6, W]; [1024, bc] -> [128, G, 8, W]
        m["xa16"] = np.ascontiguousarray(
            xac.reshape(K1T, 128, NG, W).transpose(1, 2, 0, 3))
        m["xr8"] = np.ascontiguousarray(
            xrc.reshape(KRT, 128, NG, W).transpose(1, 2, 0, 3))
        in_maps.append(m)

    res = run_bass_kernel_spmd(nc, in_maps, list(range(NCORES)), **spmd_kwargs)
    outs = [res.results[i]["out"] for i in range(NCORES)]
    return res, np.concatenate(outs, axis=0).astype(np.float32)


def kernel(**inputs):
    return run(inputs)[1]


# revision 33
# speedup vs baseline: 1.3333x; 1.0071x over previous
"""Trainium2 Bass kernel for a binarized 4-layer MLP (eval mode).

Reference computation (per row of x [B=16384, 784]):
  h1 = x @ sign(w1).T + b1;  s1 = sign(bn1(h1))        (clip doesn't change sign)
  h2 = s1 @ sign(w2).T + b2; s2 = sign(bn2(h2))
  h3 = s2 @ sign(w3).T + b3; y3 = clip(bn3(h3), -1, 1)
  z  = y3 @ w4.T + b4;       out = log_softmax(z)

Sharding: pure data-parallel over the batch across 8 NeuronCores
(weights replicated, no collectives).

Numerics (PE cost law: matmul time = out-cols cycles; DoubleRow packs
256 k-rows/instr vs 128, so fp8-DR is the only 2x lever):
  - L1: x is split ON HOST.  Rows 0-767: xa = fp16(x) (6 full 128-row
    fp16 k-tiles) plus an e4m3 residual stream e4m3((x - xa) * 2^9)
    contracted against +-2^-9 weights (exact e4m3 subnormals).  Rows
    768-783 skip fp16 entirely and ride the residual stream's pad space
    as a chain of four 4-bit e4m3 chunks (weights +-1 then +-2^-9),
    giving ~2^-16 per-element precision.  6 fp16 + 4 DR matmuls per
    m-tile replace the 14 fp16 matmuls of a two-stream fp16 split:
    final rel err 7.65e-3 (matches f64 simulation) vs 2e-2 budget.
  - L2/L3: both operands are exactly +-1/0 in fp8e4 -> DoubleRow fp8
    matmuls produce bit-exact integer sums in fp32 PSUM.
  - L4 runs transposed (w4 [128,10] stationary, h3 moving) so the PE
    streams wide moving operands instead of 10-col ones; [10, b]
    logit tiles are PE-transposed back for the row-major epilogue.
  - BN + bias folding: bn(h + b) = A*h + C with A = g*rsqrt(v+eps),
    C = A*(b - m) + beta, applied per-partition by the Sign/Identity
    activations (fp32 internally).
Scheduling notes: keep <=6 live PSUM banks (more HAM-oscillates the PE
and slows every matmul 5-20%); each dma_start costs a serialized
~650ns Sync-engine trigger, so startup transfers are few and large;
w2/w3/late-w1 streams are dependency-staged behind group-0 L1 acts.
"""

import sys

if "/opt/trn_rl_repo" not in sys.path:
    sys.path.insert(0, "/opt/trn_rl_repo")

import numpy as np

D_IN, H1, H2, H3, NCLS = 784, 3072, 1536, 768, 10
B, NCORES = 16384, 8
BC = B // NCORES          # batch rows per core
NB = 256                  # batch columns processed per chunk (L2-L4)
KF = 768                  # rows 0-767 go through the fp16 stream
K1T = KF // 128           # 6 full 128-row fp16 k-tiles
KR = 1024                 # L1 fp8 stream k rows (residuals + row chunks)
KRT = KR // 128           # 8
KRP = KRT // 2            # 4 DoubleRow pair iterations
M1, M2, M3 = H1 // 128, H2 // 128, H3 // 128   # 24, 12, 6
K2P, K3P = H1 // 256, H2 // 256                # DoubleRow k-pair iters: 12, 6
K4T = H3 // 128                                # 6
BN_EPS = 1e-5
RSC = 512.0               # fp8-stream scale 2^9; weights are +-2^-9
# const-block column layout: a1 c1 a2 c2 a3 c3 b4 id10
_C0 = [0, M1, 2 * M1, 2 * M1 + M2, 2 * (M1 + M2), 2 * (M1 + M2) + M3,
       2 * (M1 + M2 + M3), 2 * (M1 + M2 + M3) + NCLS]
NCONST = 2 * (M1 + M2 + M3) + 2 * NCLS

_cached = {}


def _build(bc):
    import concourse.bacc as bacc
    import concourse.mybir as mybir
    import concourse.tile as tile

    dt = mybir.dt
    AF = mybir.ActivationFunctionType
    PM = mybir.MatmulPerfMode
    ALU = mybir.AluOpType

    assert bc % NB == 0 and NB % 128 == 0
    gbts = bc // 128  # output row-tiles per core

    nc = bacc.Bacc("TRN2", target_bir_lowering=False, debug=False,
                   num_devices=NCORES)

    # x / w1 streams are pre-rearranged on host so every DMA moves long
    # contiguous per-partition lines:
    #   xa16 [128, G, 6, W]   (g, kt)-sliced fp16 x rows 0-767
    #   xr8  [128, G, 8, W]   e4m3 stream: scaled residuals of rows 0-767
    #                         plus 4x 4-bit chunks of rows 768-783
    #   w1t  [128, 6, 3072]   fp16 sign(w1).T rows 0-767, 1024-col chunks
    #   w1r  [128, 6, 8, 512] e4m3 fp8-stream weights in 512-col m-blocks
    NG = bc // (2 * NB) if bc % (2 * NB) == 0 else bc // NB
    WG = bc // NG
    xa16 = nc.declare_dram_parameter("xa16", [128, NG, K1T, WG], dt.float16,
                                     isOutput=False)
    xr8 = nc.declare_dram_parameter("xr8", [128, NG, KRT, WG], dt.float8e4,
                                    isOutput=False)
    w1t = nc.declare_dram_parameter("w1t", [128, K1T, H1], dt.float16,
                                    isOutput=False)
    w1r = nc.declare_dram_parameter("w1r", [128, M1 // 4, KRT, 512],
                                    dt.float8e4, isOutput=False)
    w2t = nc.declare_dram_parameter("w2t", [H1, H2], dt.float8e4, isOutput=False)
    w3t = nc.declare_dram_parameter("w3t", [H2, H3], dt.float8e4, isOutput=False)
    w4t = nc.declare_dram_parameter("w4t", [H3, NCLS], dt.bfloat16, isOutput=False)
    csts = nc.declare_dram_parameter("csts", [128, NCONST], dt.float32,
                                     isOutput=False)
    out = nc.declare_dram_parameter("out", [bc, NCLS], dt.float32, isOutput=True)

    with tile.TileContext(nc) as tc, \
            tc.tile_pool(name="wts", bufs=1) as wp, \
            tc.tile_pool(name="xin", bufs=2) as xp, \
            tc.tile_pool(name="act", bufs=2) as ap_, \
            tc.tile_pool(name="eps", bufs=2) as ep, \
            tc.tile_pool(name="ps", bufs=4, space="PSUM") as ps, \
            tc.tile_pool(name="psz", bufs=1, space="PSUM") as psz:

        # ---- startup-critical transfers first: chunk-0 x streams and the
        # first w1 chunk go out ahead of everything (small DMAs have ~1.4us
        # fixed latency each, so the consts ride in ONE packed transfer,
        # emitted after the compute-critical streams).  w2/w3 streams are
        # dependency-chained onto chunk-0 compute milestones below so they
        # don't steal HBM bandwidth at startup.

        # L1 runs on wide batch groups (W columns); L2-L4 iterate over
        # NB-column halves of each group.
        W = 2 * NB if bc % (2 * NB) == 0 else NB
        ngroups = bc // W
        halves = W // NB
        NBLK = M1 // 4            # m-tile blocks of 4 (PSUM rotation depth)

        def load_x(g):
            # returns (list of per-k-tile fp16 APs, residual [128,KRT,W] AP,
            #          first dma)
            if g == 0:
                # group 0 is startup-latency critical, but each dma_start
                # costs a serialized ~650ns Sync-engine trigger, so the six
                # k-tiles ride in two 3-k-tile transfers interleaved with
                # the two w1 chunk-0 halves (emitted by the caller).
                xas = []
                xdma = None
                for half in range(2):
                    xah = xp.tile([128, 3, W], dt.float16, tag=f"xa0{half}",
                                  bufs=1, name=f"xa0{half}")
                    d = nc.sync.dma_start(
                        xah[:], xa16.ap()[:, 0, 3 * half:3 * half + 3, :])
                    xdma = xdma or d
                    xas += [xah[:, k, :] for k in range(3)]
                    _w1_chunk0_half(half)
                xrg = xp.tile([128, KRT, W], dt.float8e4, tag="xr0", bufs=1,
                              name="xr0")
                nc.sync.dma_start(xrg[:], xr8.ap()[:, 0, :, :])
                return xas, xrg, xdma
            xag = xp.tile([128, K1T, W], dt.float16, tag="xa")
            xdma = nc.sync.dma_start(xag[:], xa16.ap()[:, g, :, :])
            xrg = xp.tile([128, KRT, W], dt.float8e4, tag="xr")
            xrdma = nc.sync.dma_start(xrg[:], xr8.ap()[:, g, :, :])
            return ([xag[:, k, :] for k in range(K1T)], xrg, xdma)

        # w1 streams in 1024-col chunks so group-0 m-tile blocks are fed
        # as they compute; chunk 0 + the first two residual blocks go out
        # up-front, later chunks are staged behind group-0 L1 progress.
        MC = 3                    # 1024-col w1 chunks
        staged = {}               # m-tile act -> dmas released by it
        w1ct = {}                 # chunk tiles: (0, half) or mc -> tile
        w1rb = [None] * NBLK

        def _w1_chunk0_half(half):
            t = wp.tile([128, 3, 1024], dt.float16, tag=f"w1c0{half}",
                        name=f"w1c0{half}")
            w1ct[(0, half)] = t
            return nc.sync.dma_start(
                t[:], w1t.ap()[:, 3 * half:3 * half + 3, 0:1024])

        def _w1_chunk(mc):
            t = wp.tile([128, K1T, 1024], dt.float16, tag=f"w1c{mc}",
                        name=f"w1c{mc}")
            w1ct[mc] = t
            return [nc.sync.dma_start(
                t[:], w1t.ap()[:, :, mc * 1024:(mc + 1) * 1024])]

        def w1ap(kt, mc):
            if mc == 0:
                return w1ct[(0, kt // 3)][:, kt % 3, :]
            return w1ct[mc][:, kt, :]

        def _w1r_block(mg):
            w1rb[mg] = wp.tile([128, KRT, 512], dt.float8e4, tag=f"w1r{mg}",
                               name=f"w1r{mg}")
            return nc.sync.dma_start(w1rb[mg][:], w1r.ap()[:, mg, :, :])

        x0 = load_x(0)
        _w1r_block(0)
        _w1r_block(1)

        cstb = wp.tile([128, NCONST], dt.float32, tag="csts")
        nc.sync.dma_start(cstb[:], csts[:])
        a1sb = cstb[:, _C0[0]:_C0[0] + M1]
        c1sb = cstb[:, _C0[1]:_C0[1] + M1]
        a2sb = cstb[:, _C0[2]:_C0[2] + M2]
        c2sb = cstb[:, _C0[3]:_C0[3] + M2]
        a3sb = cstb[:, _C0[4]:_C0[4] + M3]
        c3sb = cstb[:, _C0[5]:_C0[5] + M3]
        b4sb = cstb[:, _C0[6]:_C0[6] + NCLS]
        id10sb = cstb[0:NCLS, _C0[7]:_C0[7] + NCLS]

        w4sb = wp.tile([128, K4T, NCLS], dt.bfloat16, tag="w4")
        nc.sync.dma_start(w4sb[:], w4t.ap().rearrange("(kt p) n -> p kt n", p=128))

        staged[0] = _w1_chunk(1) + [_w1r_block(2), _w1r_block(3)]
        staged[8] = _w1_chunk(2) + [_w1r_block(4), _w1r_block(5)]

        w2sb = wp.tile([128, 2 * K2P, H2], dt.float8e4, tag="w2")
        w2_dmas = [
            nc.sync.dma_start(w2sb[:, kt, :], w2t[kt * 128:(kt + 1) * 128, :])
            for kt in range(2 * K2P)
        ]
        w3sb = wp.tile([128, 2 * K3P, H3], dt.float8e4, tag="w3")
        w3_dmas = [
            nc.sync.dma_start(w3sb[:, kt, :], w3t[kt * 128:(kt + 1) * 128, :])
            for kt in range(2 * K3P)
        ]

        zout = wp.tile([128, gbts, NCLS], dt.float32, tag="zout")
        ssum = wp.tile([128, gbts], dt.float32, tag="ssum")
        lsum = wp.tile([128, gbts], dt.float32, tag="lsum")

        def emit_epilogue(lo, hi):
            # log_softmax over the free dim; |z| is small so no max-shift
            for g in range(lo, hi):
                e = ep.tile([128, NCLS], dt.float32, tag="e")
                nc.scalar.activation(e[:], zout[:, g, :], AF.Exp,
                                     accum_out=ssum[:, g:g + 1])
            nc.scalar.activation(lsum[:, lo:hi], ssum[:, lo:hi], AF.Ln)
            for g in range(lo, hi):
                nc.vector.tensor_scalar(zout[:, g, :], zout[:, g, :],
                                        lsum[:, g:g + 1], None,
                                        op0=ALU.subtract)
            nc.sync.dma_start(
                out.ap()[lo * 128:hi * 128, :].rearrange("(g p) n -> p g n",
                                                         p=128),
                zout[:, lo:hi, :])

        prev_act0 = None
        act_g0_late = None
        for g in range(ngroups):
            xa, xrg, _ = x0 if g == 0 else (None, None, None)
            if g != 0:
                xa, xrg, xdma = load_x(g)
                # keep ~one group of x lookahead; group 1's prefetch waits
                # for late group-0 progress so it doesn't fight the
                # startup weight transfers
                gate = act_g0_late if g == 1 else prev_act0
                if gate is not None:
                    tile.add_dep_helper(xdma.ins, gate.ins, sync=True,
                                        reason="x prefetch staging")

            # ---- L1: [784 -> 3072] = 7 fp16 matmuls + 4 fp8 DoubleRow
            # residual matmuls per m-tile, blocks of 4 m-tiles over 4 PSUM
            # banks, kt-outer so the PE consumes each w1/x k-tile as its
            # DMA lands at startup.
            h1sb = ap_.tile([128, 2 * K2P, W], dt.float8e4, tag="h1")

            def l1_sign(mt, pt):
                act = nc.scalar.activation(h1sb[:, mt, :], pt[:], AF.Sign,
                                           bias=c1sb[:, mt:mt + 1],
                                           scale=a1sb[:, mt:mt + 1])
                if g == 0:
                    # stage w1-chunk/w2/w3 streams behind group-0 L1
                    # progress so they don't starve the startup transfers
                    for wd in staged.get(mt, ()):
                        tile.add_dep_helper(wd.ins, act.ins, sync=True,
                                            reason="w1 stream staging")
                    for wd_list, base in ((w2_dmas, 0), (w3_dmas, M1 // 2)):
                        for kt2, wd in enumerate(wd_list):
                            if base + kt2 // 2 == mt:
                                tile.add_dep_helper(
                                    wd.ins, act.ins, sync=True,
                                    reason="weight stream staging")
                return act

            for mg in range(NBLK):
                pts = [ps.tile([128, W], dt.float32, tag="ps",
                               name=f"pt{i}") for i in range(4)]
                for kt in range(K1T):
                    for i in range(4):
                        mt = mg * 4 + i
                        lhs = w1ap(kt, mt // 8)[:, (mt % 8) * 128:
                                                (mt % 8 + 1) * 128]
                        nc.tensor.matmul(pts[i][:], lhs, xa[kt][:],
                                         start=(kt == 0), stop=False)
                for i in range(4):
                    mt = mg * 4 + i
                    for rp in range(KRP):
                        nc.tensor.matmul(
                            pts[i][:],
                            w1rb[mg][:, 2 * rp:2 * rp + 2,
                                     i * 128:(i + 1) * 128],
                            xrg[:, 2 * rp:2 * rp + 2, :],
                            start=False, stop=(rp == KRP - 1),
                            perf_mode=PM.DoubleRow)
                    a = l1_sign(mt, pts[i])
                    if mt == 0:
                        prev_act0 = a
                    if g == 0 and mt == 20:
                        act_g0_late = a

            # all groups but the last run L2-L4 at the full W columns
            # (fewer instructions); the last group runs NB-column halves so
            # the serial L3->L4->epilogue tail telescopes on a small piece.
            HB = NB if g == ngroups - 1 else W
            for h in range(W // HB):
                hs = slice(h * HB, (h + 1) * HB)
                # ---- L2: [3072 -> 1536], fp8 DoubleRow
                h2sb = ap_.tile([128, 2 * K3P, HB], dt.float8e4,
                                tag=f"h2_{HB}")
                for mt in range(M2):
                    pt = ps.tile([128, HB], dt.float32, tag="ps")
                    for kp in range(K2P):
                        nc.tensor.matmul(
                            pt[:],
                            w2sb[:, 2 * kp:2 * kp + 2, mt * 128:(mt + 1) * 128],
                            h1sb[:, 2 * kp:2 * kp + 2, hs],
                            start=(kp == 0), stop=(kp == K2P - 1),
                            perf_mode=PM.DoubleRow)
                    nc.scalar.activation(h2sb[:, mt, :], pt[:], AF.Sign,
                                         bias=c2sb[:, mt:mt + 1],
                                         scale=a2sb[:, mt:mt + 1])

                # ---- L3: [1536 -> 768], fp8 DoubleRow; output clipped bf16
                h3c = ap_.tile([128, K4T, HB], dt.bfloat16, tag=f"h3_{HB}")
                for mt in range(M3):
                    pt = ps.tile([128, HB], dt.float32, tag="ps")
                    for kp in range(K3P):
                        nc.tensor.matmul(
                            pt[:],
                            w3sb[:, 2 * kp:2 * kp + 2, mt * 128:(mt + 1) * 128],
                            h2sb[:, 2 * kp:2 * kp + 2, :],
                            start=(kp == 0), stop=(kp == K3P - 1),
                            perf_mode=PM.DoubleRow)
                    nc.vector.tensor_scalar(h3c[:, mt, :], pt[:],
                                            a3sb[:, mt:mt + 1],
                                            c3sb[:, mt:mt + 1],
                                            op0=ALU.mult, op1=ALU.add)
                    nc.vector.tensor_scalar(h3c[:, mt, :], h3c[:, mt, :],
                                            1.0, -1.0, op0=ALU.min,
                                            op1=ALU.max)

                # ---- L4: z.T = w4 @ y3.T, w4 stationary [128,10], h3
                # moving [128,HB]; then PE-transpose [10,128] chunks back
                # to row-major and add b4.
                zp = psz.tile([NCLS, W], dt.float32, tag="zp")
                for kt in range(K4T):
                    nc.tensor.matmul(zp[:, hs], w4sb[:, kt, :], h3c[:, kt, :],
                                     start=(kt == 0), stop=(kt == K4T - 1))
                zs = ep.tile([NCLS, HB], dt.float32, tag=f"zs_{HB}")
                nc.vector.tensor_copy(zs[:], zp[:, hs])
                for bt in range(HB // 128):
                    gbt = (g * W + h * HB) // 128 + bt
                    ztp = psz.tile([128, NCLS], dt.float32, tag="zt")
                    nc.tensor.transpose(ztp[:],
                                        zs[:, bt * 128:(bt + 1) * 128],
                                        id10sb)
                    nc.vector.tensor_add(zout[:, gbt, :], ztp[:], b4sb)

                if g == ngroups - 1 and ngroups >= 2 and HB < W and h == 0:
                    # first half of the last group: epilogue overlaps the
                    # second half's matmuls
                    emit_epilogue(gbts - 4, gbts - 2)

            if g == ngroups - 2:
                # bulk of the log-softmax epilogue hides under the last
                # group's matmuls; only the final row-tiles run in the tail
                emit_epilogue(0, (g + 1) * W // 128)

        if ngroups >= 2 and halves == 2:
            # per-row-tile pieces so each DMA overlaps the next tile's chain
            emit_epilogue(gbts - 2, gbts - 1)
            emit_epilogue(gbts - 1, gbts)
        elif ngroups >= 2:
            emit_epilogue((ngroups - 1) * W // 128, gbts)
        else:
            emit_epilogue(0, gbts)

    nc.finalize()
    return nc


def _prep(x, w1, b1, w2, b2, w3, b3, w4, b4,
          g1, be1, m1, v1, g2, be2, m2, v2, g3, be3, m3, v3):
    """Host-side layout prep: transposes, binarized weight casts, BN folds,
    and the fp16 + scaled-e4m3-residual split of x."""
    import concourse.mybir as mybir
    f8 = mybir.dt.np(mybir.dt.float8e4)

    def fold(g, be, m, v, b):
        a = (g / np.sqrt(v + np.float32(BN_EPS))).astype(np.float32)
        c = (a * (b - m) + be).astype(np.float32)
        return a, c

    a1, c1 = fold(g1, be1, m1, v1, b1)
    a2, c2 = fold(g2, be2, m2, v2, b2)
    a3, c3 = fold(g3, be3, m3, v3, b3)

    def cols(v, mtiles):
        return np.ascontiguousarray(v.reshape(mtiles, 128).T)

    s1 = np.sign(w1).T.astype(np.float32)  # [784, 3072]
    NX = D_IN - KF                         # 16 leftover rows
    # fp8-stream weight rows: residual weights for rows 0-767, then the
    # leftover-row chunk weights (+-1 for chunk 0, +-2^-9 for chunks 1-3)
    w1rp = np.zeros((KR, H1), dtype=f8)
    w1rp[:KF] = (s1[:KF] / np.float32(RSC)).astype(f8)
    w1rp[KF:KF + NX] = s1[KF:].astype(f8)
    for j in range(1, 4):
        w1rp[KF + j * NX:KF + (j + 1) * NX] = (s1[KF:] / np.float32(RSC)).astype(f8)
    # [1024, 3072] -> [128, 6 m-blocks, 8 k-tiles, 512]
    w1r4 = np.ascontiguousarray(
        w1rp.reshape(KRT, 128, M1 // 4, 512).transpose(1, 2, 0, 3))
    # [768, 3072] -> [128, 6 k-tiles, 3072]
    w1t3 = np.ascontiguousarray(
        s1[:KF].astype(np.float16).reshape(K1T, 128, H1).transpose(1, 0, 2))

    cst = np.zeros((128, NCONST), dtype=np.float32)
    for i, v in enumerate((cols(a1, M1), cols(c1, M1), cols(a2, M2),
                           cols(c2, M2), cols(a3, M3), cols(c3, M3),
                           np.tile(b4.astype(np.float32), (128, 1)))):
        cst[:, _C0[i]:_C0[i] + v.shape[1]] = v
    cst[:NCLS, _C0[7]:_C0[7] + NCLS] = np.eye(NCLS, dtype=np.float32)

    pre = dict(
        w1t=w1t3,
        w1r=w1r4,
        w2t=np.ascontiguousarray(np.sign(w2).T).astype(f8),
        w3t=np.ascontiguousarray(np.sign(w3).T).astype(f8),
        w4t=np.ascontiguousarray(w4.T).astype(mybir.dt.np(mybir.dt.bfloat16)),
        csts=cst,
    )
    xt = np.ascontiguousarray(x.T.astype(np.float32))  # [784, B]
    xa = xt[:KF].astype(np.float16)
    xr = np.zeros((KR, x.shape[0]), dtype=f8)
    xr[:KF] = ((xt[:KF] - xa.astype(np.float32)) * np.float32(RSC)).astype(f8)
    # leftover rows 768-783: chain of four 4-bit e4m3 chunks
    r = xt[KF:]
    for j in range(4):
        s = np.float32(1.0 if j == 0 else RSC)
        q = (r * s).astype(f8)
        xr[KF + j * NX:KF + (j + 1) * NX] = q
        r = r - q.astype(np.float32) / s
    return pre, xa, xr


def run(inputs, **spmd_kwargs):
    from concourse.bass_utils import run_bass_kernel_spmd

    if "nc" not in _cached:
        _cached["nc"] = _build(BC)
    nc = _cached["nc"]

    inputs = {k: np.asarray(v) for k, v in inputs.items()}
    pre, xa, xr = _prep(**inputs)

    W = 2 * NB if BC % (2 * NB) == 0 else NB
    NG = BC // W
    in_maps = []
    for core in range(NCORES):
        m = dict(pre)
        xac = xa[:, core * BC:(core + 1) * BC]
        xrc = xr[:, core * BC:(core + 1) * BC]
        # [768, bc] -> [128, G, 6, W]; [1024, bc] -> [128, G, 8, W]
        m["xa16"] = np.ascontiguousarray(
            xac.reshape(K1T, 128, NG, W).transpose(1, 2, 0, 3))
        m["xr8"] = np.ascontiguousarray(
            xrc.reshape(KRT, 128, NG, W).transpose(1, 2, 0, 3))
        in_maps.append(m)

    res = run_bass_kernel_spmd(nc, in_maps, list(range(NCORES)), **spmd_kwargs)
    outs = [res.results[i]["out"] for i in range(NCORES)]
    return res, np.concatenate(outs, axis=0).astype(np.float32)


def kernel(**inputs):
    return run(inputs)[1]
